# revision 14
# baseline (speedup 1.0000x reference)
"""CrossAttention (LayerNorm + 16-head cross-attention + out-proj) on 8 TRN2 cores.

Sharding: 8 cores = 4 batches x 2 head-groups (8 heads each).
Each core computes its batch's LayerNorm + Q/K/V projections for its head
group, the attention, and a partial output projection (row-slice of Wo);
the host sums the two partials per batch at unshard time.

Device compute layout is fully transposed ("T layout"): activations are
kept [feature, token] so every matmul contraction runs over the partition
dim at full fp32r rate (1 cycle/row).  Softmax runs without max-subtraction
(dots are O(6) here), so the kv-partition layout needs no partition
reductions: row-sums ride along the PV matmul as an extra ones-column of v.
"""

import sys

sys.path.insert(0, "/opt/trn_rl_repo")

import numpy as np

import concourse.bacc as bacc
import concourse.bass as bass
import concourse.mybir as mybir
import concourse.tile as tile
from concourse.bass_utils import run_bass_kernel_spmd

P = 128
F32 = mybir.dt.float32
F32R = mybir.dt.float32r
U8 = mybir.dt.uint8
AF = mybir.ActivationFunctionType


class Cfg:
    def __init__(self, B=4, NQ=1024, NKV=2048, D=1024, HEADS=16, DH=64, HG=2):
        self.B, self.NQ, self.NKV, self.D = B, NQ, NKV, D
        self.HEADS, self.DH, self.HG = HEADS, DH, HG
        self.HPG = HEADS // HG              # heads per group (per core)
        self.IG = self.HPG * DH             # inner slice per core
        self.NT_D = D // P
        self.NT_I = self.IG // P
        self.NT_KV = NKV // P
        self.NT_Q = NQ // P
        self.EPS = 1e-5
        self.SCALE = DH ** -0.5
        self.n_cores = B * HG


def fsplit(n):
    """Split free dim n into <=512 chunks (fp32 moving-operand limit)."""
    return [(j, min(512, n - j)) for j in range(0, n, 512)]


def build_nc(cfg: Cfg, dbg=False):
    c = cfg
    nc = bacc.Bacc("TRN2", target_bir_lowering=False, debug=False)

    xT = nc.dram_tensor("xT", [c.D, c.NQ], F32R, kind="ExternalInput").ap()
    keyT = nc.dram_tensor("keyT", [c.D, c.NKV], F32R, kind="ExternalInput").ap()
    valueT = nc.dram_tensor("valueT", [c.D, c.NKV], F32R, kind="ExternalInput").ap()
    maskKT = nc.dram_tensor("maskKT", [c.NKV, c.NQ], U8, kind="ExternalInput").ap()
    wq_d = nc.dram_tensor("wq", [c.D, c.IG], F32R, kind="ExternalInput").ap()
    wk_d = nc.dram_tensor("wk", [c.D, c.IG], F32R, kind="ExternalInput").ap()
    wv_d = nc.dram_tensor("wv", [c.D, c.IG], F32R, kind="ExternalInput").ap()
    wo_d = nc.dram_tensor("wo", [c.IG, c.D], F32R, kind="ExternalInput").ap()
    exq_d = nc.dram_tensor("exq", [2, c.IG], F32R, kind="ExternalInput").ap()
    out_d = nc.dram_tensor("out_part", [c.NQ, c.D], F32, kind="ExternalOutput").ap()
    if dbg:
        dq = nc.dram_tensor("dbg_qT", [P, c.NT_I * c.NQ], F32, kind="ExternalOutput").ap()
        dk = nc.dram_tensor("dbg_kt", [P, c.NT_I * c.NKV], F32, kind="ExternalOutput").ap()
        dv = nc.dram_tensor("dbg_vaug", [P, c.NT_KV * c.HPG * (c.DH + 1)], F32, kind="ExternalOutput").ap()
        de = nc.dram_tensor("dbg_et", [P, c.NQ], F32, kind="ExternalOutput").ap()
        do = nc.dram_tensor("dbg_oT", [P, c.NT_I * c.NQ], F32, kind="ExternalOutput").ap()
        dr = nc.dram_tensor("dbg_rhsx", [2, c.NQ], F32, kind="ExternalOutput").ap()
        drs = nc.dram_tensor("dbg_rs", [1, c.NQ], F32, kind="ExternalOutput").ap()

    with tile.TileContext(nc) as tc:
        with (
            tc.tile_pool(name="persist", bufs=1) as pp,
            tc.tile_pool(name="wts", bufs=2) as wp,
            tc.tile_pool(name="stream", bufs=2) as sp,
            tc.tile_pool(name="stat", bufs=1) as stp,
            tc.tile_pool(name="exp", bufs=4) as ep,
            tc.tile_pool(name="rec", bufs=1) as rp,
            tc.tile_pool(name="outp", bufs=2) as op_,
            tc.tile_pool(name="ps", bufs=4, space="PSUM") as ps,
        ):
            # ---- resident tiles ----
            xt = pp.tile([P, c.NT_D, c.NQ], F32R, tag="xt")
            kt = pp.tile([P, c.NT_I, c.NKV], F32R, tag="kt")
            vab = c.HPG * (c.DH + 1)
            v_aug = pp.tile([P, c.NT_KV, vab], F32R, tag="vaug")
            exq = pp.tile([2, c.IG], F32R, tag="exq")
            ones_sq = pp.tile([P, P], F32R, tag="ones")
            rhs_x = pp.tile([2, c.NQ], F32R, tag="rhsx")
            mk = pp.tile([P, c.NT_KV, c.NQ], U8, tag="mk")

            nc.sync.dma_start(xt[:], xT.rearrange("(t p) n -> p t n", p=P))
            nc.sync.dma_start(exq[:], exq_d[:])
            # memset cannot write f32r; build constants as in0*0 + 1 instead
            nc.vector.tensor_scalar(
                ones_sq[:], xt[:, 0, 0:P].bitcast(F32), 0.0, 1.0,
                mybir.AluOpType.mult, mybir.AluOpType.add,
            )

            # ---- LayerNorm stats over feature dim (on partitions) ----
            # ones[128,128] stationary => every output partition carries the
            # full column sum, so downstream stat math runs at full width and
            # A/B arrive pre-broadcast.
            ps_s = ps.tile([P, c.NQ], F32, tag="pmm", name="ps_s")
            ps_s2 = ps.tile([P, c.NQ], F32, tag="pmm", name="ps_s2")
            for t in range(c.NT_D):
                sq = ep.tile([P, c.NQ], F32R, tag="et", name=f"sq{t}")
                nc.vector.tensor_mul(
                    sq[:], xt[:, t, :].bitcast(F32), xt[:, t, :].bitcast(F32)
                )
                for j, w in fsplit(c.NQ):
                    nc.tensor.matmul(
                        ps_s[:, j : j + w], ones_sq[:], xt[:, t, j : j + w],
                        start=(t == 0), stop=(t == c.NT_D - 1),
                    )
                    nc.tensor.matmul(
                        ps_s2[:, j : j + w], ones_sq[:], sq[:, j : j + w],
                        start=(t == 0), stop=(t == c.NT_D - 1),
                    )

            s_mu = stp.tile([P, c.NQ], F32, tag="smu")
            s_a = stp.tile([P, c.NQ], F32, tag="sa")
            s_b = stp.tile([P, c.NQ], F32, tag="sb")
            inv_d = 1.0 / c.D
            inv_s2 = 1.0 / (c.SCALE * c.SCALE)  # = DH
            nc.vector.tensor_scalar_mul(s_mu[:], ps_s[:], inv_d)
            nc.vector.tensor_scalar_mul(s_b[:], ps_s2[:], inv_d)
            nc.vector.tensor_mul(s_a[:], s_mu[:], s_mu[:])
            nc.vector.tensor_sub(s_b[:], s_b[:], s_a[:])  # var
            # sd = sqrt((var+eps)/scale^2)  =>  1/sd = scale*rstd = A
            eps_t = stp.tile([P, 1], F32, tag="eps")
            nc.vector.memset(eps_t[:], float(c.EPS * inv_s2))
            nc.scalar.activation(
                s_a[:], s_b[:], AF.Sqrt, scale=float(inv_s2), bias=eps_t[:]
            )
            nc.vector.reciprocal(s_b[:], s_a[:])          # A
            nc.vector.tensor_mul(s_a[:], s_mu[:], s_b[:])  # B = mu*A

            # extra contraction rows for Q: row0 = B (pairs with -csum),
            # row1 = 1.0 (pairs with beta@Wq)
            nc.vector.tensor_scalar(
                rhs_x[:], s_a[0:2, :], 0.0, 1.0,
                mybir.AluOpType.mult, mybir.AluOpType.add,
            )
            nc.vector.tensor_copy(rhs_x[0:1, :], s_a[0:1, :])

            # ---- prescale: xs = xT * A (in place) ----
            for t in range(c.NT_D):
                nc.vector.tensor_mul(
                    xt[:, t, :], xt[:, t, :].bitcast(F32), s_b[:]
                )

            # qT/oT live in the xt slot (xt is dead once Q-proj reads finish)
            qo = pp.tile([P, 2 * c.NT_I, c.NQ], F32R, tag="xt", name="qo")
            qT = qo[:, 0 : c.NT_I, :]
            oT = qo[:, c.NT_I : 2 * c.NT_I, :]

            # ---- Q projection: qT[i, n] = sum_d wq[d, i] * xs[d, n] + extras
            wq = wp.tile([P, c.NT_D, c.IG], F32R, tag="w")
            nc.sync.dma_start(wq[:], wq_d.rearrange("(t p) m -> p t m", p=P))
            for m in range(c.NT_I):
                pq = ps.tile([P, c.NQ], F32, tag="pmm")
                for t in range(c.NT_D):
                    for j, w in fsplit(c.NQ):
                        nc.tensor.matmul(
                            pq[:, j : j + w],
                            wq[:, t, m * P : (m + 1) * P],
                            xt[:, t, j : j + w],
                            start=(t == 0), stop=False,
                        )
                for j, w in fsplit(c.NQ):
                    nc.tensor.matmul(
                        pq[:, j : j + w],
                        exq[:, m * P : (m + 1) * P],
                        rhs_x[:, j : j + w],
                        start=False, stop=True,
                    )
                nc.vector.tensor_copy(qT[:, m, :], pq[:])

            # ---- K projection: kT[i, kv] ----
            wk = wp.tile([P, c.NT_D, c.IG], F32R, tag="w")
            nc.sync.dma_start(wk[:], wk_d.rearrange("(t p) m -> p t m", p=P))
            keyT_r = keyT.rearrange("(t p) n -> p t n", p=P)
            for h0 in range(0, c.NKV, 1024):
                hw = min(1024, c.NKV - h0)
                pks = [ps.tile([P, hw], F32, tag="pmm", name=f"pk_{h0}_{m}")
                       for m in range(c.NT_I)]
                for t in range(c.NT_D):
                    ktile = sp.tile([P, 1024], F32R, tag="st")
                    nc.sync.dma_start(ktile[:, :hw], keyT_r[:, t, h0 : h0 + hw])
                    for m in range(c.NT_I):
                        for j, w in fsplit(hw):
                            nc.tensor.matmul(
                                pks[m][:, j : j + w],
                                wk[:, t, m * P : (m + 1) * P],
                                ktile[:, j : j + w],
                                start=(t == 0), stop=(t == c.NT_D - 1),
                            )
                for m in range(c.NT_I):
                    nc.vector.tensor_copy(kt[:, m, h0 : h0 + hw], pks[m][:])

            # ---- V projection: v[kv, i] row-major, augmented ones column ----
            wv = wp.tile([P, c.NT_D, c.IG], F32R, tag="w")
            nc.sync.dma_start(wv[:], wv_d.rearrange("(t p) m -> p t m", p=P))
            valueT_r = valueT.rearrange("(t p) n -> p t n", p=P)
            ig_split = fsplit(c.IG)
            for h0 in range(0, c.NKV, 1024):
                hw = min(1024, c.NKV - h0)
                nmk = hw // P
                pvs = [ps.tile([P, 1024], F32, tag="pmm", name=f"pv_{h0}_{m}")
                       for m in range((nmk + 1) // 2)]
                for t in range(c.NT_D):
                    vtile = sp.tile([P, 1024], F32R, tag="st")
                    nc.sync.dma_start(vtile[:, :hw], valueT_r[:, t, h0 : h0 + hw])
                    for mk_ in range(nmk):
                        pv = pvs[mk_ // 2]
                        off = (mk_ % 2) * 512
                        assert c.IG <= 512
                        nc.tensor.matmul(
                            pv[:, off : off + c.IG],
                            vtile[:, mk_ * P : (mk_ + 1) * P],
                            wv[:, t, :],
                            start=(t == 0), stop=(t == c.NT_D - 1),
                        )
                for mk_ in range(nmk):
                    kvt = h0 // P + mk_
                    pv = pvs[mk_ // 2]
                    off = (mk_ % 2) * 512
                    dst = v_aug[:, kvt, :].rearrange("p (h e) -> p h e", e=c.DH + 1)
                    nc.vector.tensor_copy(
                        dst[:, :, 0 : c.DH],
                        pv[:, off : off + c.IG].rearrange("p (h d) -> p h d", d=c.DH),
                    )
                    nc.vector.tensor_scalar(
                        dst[:, :, c.DH : c.DH + 1],
                        pv[:, off : off + c.HPG],
                        0.0, 1.0,
                        mybir.AluOpType.mult, mybir.AluOpType.add,
                    )

            # ---- mask (loaded late so the DMA doesn't hog SBUF early) ----
            nc.sync.dma_start(mk[:], maskKT.rearrange("(t p) n -> p t n", p=P))
            wo = wp.tile([P, c.NT_I, c.D], F32R, tag="w", name="wo")
            nc.sync.dma_start(wo[:], wo_d.rearrange("(t p) m -> p t m", p=P))

            # ---- attention per head ----
            for h in range(c.HPG):
                th, po = (h * c.DH) // P, (h * c.DH) % P
                pso = ps.tile([c.DH + 1, c.NQ], F32, tag="pmm")
                for kvt in range(c.NT_KV):
                    psd = ps.tile([P, c.NQ], F32, tag="pmm")
                    for j, w in fsplit(c.NQ):
                        nc.tensor.matmul(
                            psd[:, j : j + w],
                            kt[po : po + c.DH, th, kvt * P : (kvt + 1) * P],
                            qT[po : po + c.DH, th, j : j + w],
                            start=True, stop=True,
                        )
                    et = ep.tile([P, c.NQ], F32R, tag="et")
                    nc.scalar.activation(et[:], psd[:], AF.Exp)
                    nc.vector.tensor_mul(et[:], et[:].bitcast(F32), mk[:, kvt, :])
                    if dbg and h == 0 and kvt == 0:
                        nc.sync.dma_start(de[:], et[:].bitcast(F32))
                    for j, w in fsplit(c.NQ):
                        nc.tensor.matmul(
                            pso[:, j : j + w],
                            v_aug[:, kvt, h * (c.DH + 1) : (h + 1) * (c.DH + 1)],
                            et[:, j : j + w],
                            start=(kvt == 0), stop=(kvt == c.NT_KV - 1),
                        )
                # row DH of pso is the exp row-sum.  Broadcast it to DH
                # partitions with a K=1 matmul (ones[1,DH] stationary), then
                # reciprocal + multiply, all partition-aligned; only the final
                # DMA moves data across partitions.
                rs65 = rp.tile([c.DH + 1, c.NQ], F32R, tag="rsr")
                nc.vector.tensor_copy(
                    rs65[c.DH : c.DH + 1, :], pso[c.DH : c.DH + 1, :]
                )
                if dbg and h == 0:
                    nc.sync.dma_start(drs[:], rs65[c.DH : c.DH + 1, :].bitcast(F32))
                prec = ps.tile([c.DH, c.NQ], F32, tag="pmm", name=f"prec{h}")
                for j, w in fsplit(c.NQ):
                    nc.tensor.matmul(
                        prec[:, j : j + w],
                        ones_sq[c.DH : c.DH + 1, 0 : c.DH],
                        rs65[c.DH : c.DH + 1, j : j + w],
                        start=True, stop=True,
                    )
                rec = rp.tile([c.DH, c.NQ], F32, tag="rec")
                nc.vector.reciprocal(rec[:], prec[:])
                if po == 0:
                    nc.vector.tensor_mul(
                        oT[0 : c.DH, th, :], pso[0 : c.DH, :], rec[:]
                    )
                else:
                    otmp = rp.tile([c.DH, c.NQ], F32R, tag="otmp")
                    nc.vector.tensor_mul(otmp[:], pso[0 : c.DH, :], rec[:])
                    nc.sync.dma_start(oT[po : po + c.DH, th, :], otmp[:])

            if dbg:
                nc.sync.dma_start(dq[:], qT.bitcast(F32))
                nc.sync.dma_start(dk[:], kt[:].bitcast(F32))
                nc.sync.dma_start(dv[:], v_aug[:].bitcast(F32))
                nc.sync.dma_start(do[:], oT.bitcast(F32))
                nc.sync.dma_start(dr[:], rhs_x[:].bitcast(F32))

            # ---- output projection (partial: this core's Wo row-slice) ----
            for mq in range(c.NT_Q):
                pr = ps.tile([P, c.D], F32, tag="pmm")
                for t in range(c.NT_I):
                    for j, w in fsplit(c.D):
                        nc.tensor.matmul(
                            pr[:, j : j + w],
                            oT[:, t, mq * P : (mq + 1) * P],
                            wo[:, t, j : j + w],
                            start=(t == 0), stop=(t == c.NT_I - 1),
                        )
                rt = op_.tile([P, c.D], F32, tag="rt")
                nc.vector.tensor_copy(rt[:], pr[:])
                nc.sync.dma_start(out_d[mq * P : (mq + 1) * P, :], rt[:])

    nc.compile()
    return nc


def host_prep(cfg: Cfg, x, key_t, value, mask, Wq, Wk, Wv, Wo, gamma, beta):
    """Build per-core input maps (numpy only)."""
    c = cfg
    f = np.float32
    Wq_g = (gamma.astype(f)[:, None] * Wq.astype(f))
    bq = (beta.astype(f) @ Wq.astype(f)) * c.SCALE
    in_maps = []
    cores = []
    for b in range(c.B):
        xTb = np.ascontiguousarray(x[b].astype(f).T)
        keyTb = np.ascontiguousarray(key_t[b].astype(f).T)
        valTb = np.ascontiguousarray(value[b].astype(f).T)
        mkb = np.ascontiguousarray((~mask[b].astype(bool)).T.astype(np.uint8))
        for hg in range(c.HG):
            sl = slice(hg * c.IG, (hg + 1) * c.IG)
            wq_s = np.ascontiguousarray(Wq_g[:, sl])
            exq = np.stack([-wq_s.sum(axis=0), bq[sl]]).astype(f)
            in_maps.append({
                "xT": xTb, "keyT": keyTb, "valueT": valTb, "maskKT": mkb,
                "wq": wq_s,
                "wk": np.ascontiguousarray(Wk.astype(f)[:, sl]),
                "wv": np.ascontiguousarray(Wv.astype(f)[:, sl]),
                "wo": np.ascontiguousarray(Wo.astype(f)[sl, :]),
                "exq": exq,
            })
            cores.append((b, hg))
    return in_maps, cores


_CACHE = {}


def get_nc(cfg: Cfg):
    key = tuple(sorted(cfg.__dict__.items()))
    if key not in _CACHE:
        _CACHE[key] = build_nc(cfg)
    return _CACHE[key]


def run(cfg: Cfg, inputs, trace=False):
    nc = get_nc(cfg)
    in_maps, cores = host_prep(cfg, **inputs)
    res = run_bass_kernel_spmd(
        nc, in_maps, list(range(cfg.n_cores)), trace=trace
    )
    out = np.zeros((cfg.B, cfg.NQ, cfg.D), np.float32)
    for i, (b, hg) in enumerate(cores):
        out[b] += res.results[i]["out_part"]
    return out, res


def kernel(x, key_t, value, mask, Wq, Wk, Wv, Wo, gamma, beta):
    cfg = Cfg()
    out, _ = run(cfg, dict(
        x=np.asarray(x), key_t=np.asarray(key_t), value=np.asarray(value),
        mask=np.asarray(mask), Wq=np.asarray(Wq), Wk=np.asarray(Wk),
        Wv=np.asarray(Wv), Wo=np.asarray(Wo), gamma=np.asarray(gamma),
        beta=np.asarray(beta),
    ))
    return out
